# revision 2
# baseline (speedup 1.0000x reference)
"""Trainium2 Bass kernel for nn_MultiHeadGraphAttention (v3: sorted staircase).

One head per core. Host sorts rows i by s_i and columns j by d_j (pure
layout permutations, applied to the mask / h uploads and undone on the
output). With both axes sorted, sign(s_i + d_j) forms a monotone staircase:
per j-chunk c there are compile-time column boundaries LO_c >= HI_c' ...
    cols [0, LO_c)      : t < 0 for every j in the chunk  (branch 2)
    cols [LO_c, HI_c)   : mixed "band" (~10% of the matrix)
    cols [HI_c, N)      : t >= 0 for every j               (branch 1)

Factoring u_i = exp(s_i) out of every branch (it cancels in the softmax):
    P = u_i * M * max(v_j, w_i * v2_j),  v = exp(d), v2 = exp(0.2 d),
                                         w_i = exp(-0.8 s_i)
    pure-1: P/u = v_j * M        -> PE matmul (V*v  stationary, fp8 mask moving)
    pure-2: P/u = w_i * v2_j * M -> PE matmul (V*v2 stationary), w_i applied
                                    per-column at combine time
    band:   Q = M * max(v_j, w_i v2_j) -> tiny elementwise + PE matmul
so the mask multiply + exp work for 90% of the matrix is done BY THE PE
(fp8 moving operand measured exact and full speed), DVE only touches the
band. out = (ACC1 + w*ACC2)[0:64] / (ACC1 + w*ACC2)[64] column-wise; the
ones-column of each stationary supplies the denominators; u_i cancels.

PSUM holds ACC1+ACC2 for a 1024-column quarter (8 banks); the i-range is
processed in 4 quarters, tail (combine -> transpose -> normalize -> DMA)
of quarter q overlapping the j-loop of quarter q+1.
"""
import sys

if "/opt/trn_rl_repo" not in sys.path:
    sys.path.insert(0, "/opt/trn_rl_repo")

from contextlib import ExitStack

import ml_dtypes
import numpy as np

import concourse.bass as bass
import concourse.bacc as bacc
import concourse.tile as tile
from concourse import mybir
from concourse.bass_utils import run_bass_kernel_spmd

F32 = mybir.dt.float32
BF16 = mybir.dt.bfloat16
FP8 = mybir.dt.float8e4
AF = mybir.ActivationFunctionType
ALU = mybir.AluOpType

N = 4096
F_IN = 256
N_HEAD = 8
F_OUT = 64
NEG = 0.2
NCH = N // 128        # 32 j-chunks
FC = F_IN // 128      # 2 f-chunks
VW = F_OUT + 1        # 65: V columns + ones column
IW = 1024             # i-quarter width
NQ = N // IW          # 4 quarters
BWMAX = 640           # max band width per chunk (assert against actual)


def _compute_layout(h, w, a_src, a_dst):
    """Per-head sort permutations and union staircase boundaries."""
    h64 = np.asarray(h, np.float64)
    perms = []
    los = np.zeros((N_HEAD, NCH), np.int64)
    his = np.zeros((N_HEAD, NCH), np.int64)
    for c in range(N_HEAD):
        wa_s = (np.asarray(w[c], np.float64) @ np.asarray(a_src[c], np.float64))[:, 0]
        wa_d = (np.asarray(w[c], np.float64) @ np.asarray(a_dst[c], np.float64))[:, 0]
        s = h64 @ wa_s
        d = h64 @ wa_d
        pi = np.argsort(s, kind="stable")
        pj = np.argsort(d, kind="stable")
        ss = s[pi]
        ds = d[pj]
        for jc in range(NCH):
            dmax = ds[jc * 128 + 127]
            dmin = ds[jc * 128]
            los[c, jc] = np.searchsorted(ss, -dmax, side="left")
            his[c, jc] = np.searchsorted(ss, -dmin, side="left")
        perms.append((pi, pj))
    LO = los.min(axis=0)
    HI = his.max(axis=0)
    assert np.all(np.diff(LO) <= 0) and np.all(np.diff(HI) <= 0)
    assert np.all(HI - LO <= BWMAX), f"band too wide: {int((HI-LO).max())}"
    return perms, tuple(int(x) for x in LO), tuple(int(x) for x in HI)


def build_program(LO, HI, add_b):
    nc = bacc.Bacc("TRN2", target_bir_lowering=False, debug=False)
    hTi = nc.dram_tensor("hTi", [F_IN, N], BF16, kind="ExternalInput").ap()
    hTj = nc.dram_tensor("hTj", [F_IN, N], BF16, kind="ExternalInput").ap()
    wa6 = nc.dram_tensor("wa6", [F_IN, 6], BF16, kind="ExternalInput").ap()
    w_bf = nc.dram_tensor("w_bf", [F_IN, F_OUT], BF16, kind="ExternalInput").ap()
    eye3 = nc.dram_tensor("eye3", [3, 3], F32, kind="ExternalInput").ap()
    eye65 = nc.dram_tensor("eye65", [VW, VW], F32, kind="ExternalInput").ap()
    ones1 = nc.dram_tensor("ones1", [1, 128], BF16, kind="ExternalInput").ap()
    b_row = nc.dram_tensor("b_row", [1, F_OUT], BF16, kind="ExternalInput").ap()
    mask8 = nc.dram_tensor("mask8", [N, N], FP8, kind="ExternalInput").ap()
    out_nat = nc.dram_tensor("out_nat", [N, F_OUT], F32, kind="ExternalOutput").ap()
    s_dram = nc.dram_tensor("s_scratch", [N], F32).ap()

    with tile.TileContext(nc) as tc, ExitStack() as ctx:
        const_pool = ctx.enter_context(tc.tile_pool(name="const", bufs=1))
        mask_pool = ctx.enter_context(tc.tile_pool(name="mask", bufs=40))
        tail_pool = ctx.enter_context(tc.tile_pool(name="tail", bufs=2))
        band_pool = ctx.enter_context(tc.tile_pool(name="band", bufs=3))
        sm_pool = ctx.enter_context(tc.tile_pool(name="sm", bufs=4))
        pre_ctx = ExitStack()
        psw_pool = pre_ctx.enter_context(tc.tile_pool(name="psw", bufs=2, space="PSUM"))
        pre_pool = pre_ctx.enter_context(tc.tile_pool(name="pre", bufs=1))

        # ---------------- input loads ----------------
        hTi_sb = pre_pool.tile([128, FC * N], BF16, tag="hTi")
        hTj_sb = pre_pool.tile([128, FC * N], BF16, tag="hTj")
        H2 = N // 2
        for half in range(2):
            for fc in range(FC):
                nc.sync.dma_start(
                    hTi_sb[:, fc * N + half * H2: fc * N + (half + 1) * H2],
                    hTi[fc * 128:(fc + 1) * 128, half * H2:(half + 1) * H2])
        for half in range(2):
            for fc in range(FC):
                nc.sync.dma_start(
                    hTj_sb[:, fc * N + half * H2: fc * N + (half + 1) * H2],
                    hTj[fc * 128:(fc + 1) * 128, half * H2:(half + 1) * H2])
        wa_sb = const_pool.tile([128, FC * 6], BF16, tag="wa")
        for fc in range(FC):
            nc.sync.dma_start(wa_sb[:, fc * 6:(fc + 1) * 6],
                              wa6[fc * 128:(fc + 1) * 128, :])
        w_sb = const_pool.tile([128, FC * F_OUT], BF16, tag="w")
        for fc in range(FC):
            nc.sync.dma_start(w_sb[:, fc * F_OUT:(fc + 1) * F_OUT],
                              w_bf[fc * 128:(fc + 1) * 128, :])
        eye3_sb = const_pool.tile([3, 3], F32, tag="eye3")
        nc.sync.dma_start(eye3_sb[:, :], eye3[:, :])
        eye65_sb = const_pool.tile([VW, VW], F32, tag="eye65")
        nc.sync.dma_start(eye65_sb[:, :], eye65[:, :])
        ones1_sb = const_pool.tile([1, 128], BF16, tag="ones1")
        nc.sync.dma_start(ones1_sb[:, :], ones1[:, :])
        brow_sb = const_pool.tile([1, F_OUT], BF16, tag="brow")
        nc.sync.dma_start(brow_sb[:, :], b_row[:, :])
        zst_sb = const_pool.tile([1, VW], BF16, tag="zst")
        nc.gpsimd.memset(zst_sb[:, :], 0.0)
        zmov_sb = const_pool.tile([1, 512], BF16, tag="zmov")
        nc.gpsimd.memset(zmov_sb[:, :], 0.0)

        # mask tiles: per (half, chunk) [128, 2048] fp8
        m_tiles = {}

        def issue_mask(half, jc):
            t = mask_pool.tile([128, N // 2], FP8, tag="m8", name=f"m8_{half}_{jc}")
            eng = nc.sync if jc % 2 == 0 else nc.scalar
            eng.dma_start(t[:, :], mask8[jc * 128:(jc + 1) * 128,
                                         half * H2:(half + 1) * H2])
            m_tiles[(half, jc)] = t

        for jc in range(8):
            issue_mask(0, jc)

        # ---------------- s row (from hTi) ----------------
        srow_sb = pre_pool.tile([1, N], F32, tag="srow")
        for sl in range(8):
            ps_s = psw_pool.tile([1, 512], F32, tag="pss")
            combos = [(fc, hl) for fc in range(FC) for hl in range(2)]
            for ci, (fc, hl) in enumerate(combos):
                nc.tensor.matmul(ps_s[:, :],
                                 wa_sb[:, fc * 6 + 4 + hl: fc * 6 + 5 + hl],
                                 hTi_sb[:, fc * N + sl * 512: fc * N + (sl + 1) * 512],
                                 start=(ci == 0), stop=(ci == len(combos) - 1))
            nc.vector.tensor_copy(srow_sb[0:1, sl * 512:(sl + 1) * 512], ps_s[:, :])
        # broadcast s to partitions, then W = exp(-0.8 s) table
        S_b = pre_pool.tile([128, N], F32, tag="Sb")
        for half in range(2):
            hs = slice(half * H2, (half + 1) * H2)
            nc.sync.dma_start(s_dram[hs], srow_sb[0:1, hs])
            nc.sync.dma_start(S_b[:, hs], s_dram[None, hs].broadcast_to((128, H2)))
        W_bb = const_pool.tile([128, N], BF16, tag="Wbb")
        nc.scalar.activation(W_bb[:, :], S_b[:, :], AF.Exp, scale=-0.8)

        # ---------------- d rows (from hTj) + transposes ----------------
        ddT_sb = pre_pool.tile([2, N], F32, tag="ddT")
        for sl in range(8):
            ps_d = psw_pool.tile([2, 512], F32, tag="psd")
            combos = [(fc, hl) for fc in range(FC) for hl in range(2)]
            for ci, (fc, hl) in enumerate(combos):
                nc.tensor.matmul(ps_d[:, :],
                                 wa_sb[:, fc * 6 + 2 * hl: fc * 6 + 2 * hl + 2],
                                 hTj_sb[:, fc * N + sl * 512: fc * N + (sl + 1) * 512],
                                 start=(ci == 0), stop=(ci == len(combos) - 1))
            nc.vector.tensor_copy(ddT_sb[0:2, sl * 512:(sl + 1) * 512], ps_d[:, :])
        d_sb = const_pool.tile([128, 2 * NCH], F32, tag="d")
        for jc in range(NCH):
            ps_td = psw_pool.tile([128, 2], F32, tag="pstd")
            nc.tensor.transpose(ps_td[:, :], ddT_sb[0:2, jc * 128:(jc + 1) * 128],
                                eye3_sb[0:2, 0:2])
            nc.vector.tensor_copy(d_sb[:, 2 * jc: 2 * jc + 2], ps_td[:, :])
        v_sb = const_pool.tile([128, 2 * NCH], F32, tag="v")
        nc.scalar.activation(v_sb[:, :], d_sb[:, :], AF.Exp)

        # ---------------- V (h_prime + b) and scaled stationaries ----------------
        Vb_sb = const_pool.tile([128, NCH * VW], BF16, tag="Vb")
        V1_sb = const_pool.tile([128, NCH * VW], BF16, tag="V1")
        V2_sb = const_pool.tile([128, NCH * VW], BF16, tag="V2")
        nc.vector.memset(Vb_sb[:, :], 1.0)
        for jc in range(NCH):
            ps_v = psw_pool.tile([128, F_OUT], F32, tag="psv")
            for fc in range(FC):
                nc.tensor.matmul(
                    ps_v[:, :],
                    hTj_sb[:, fc * N + jc * 128: fc * N + (jc + 1) * 128],
                    w_sb[:, fc * F_OUT:(fc + 1) * F_OUT],
                    start=(fc == 0), stop=(not add_b and fc == FC - 1))
            if add_b:
                nc.tensor.matmul(ps_v[:, :], ones1_sb[:, :], brow_sb[:, :],
                                 start=False, stop=True)
            if jc % 2 == 0:
                nc.scalar.copy(Vb_sb[:, jc * VW: jc * VW + F_OUT], ps_v[:, :])
            else:
                nc.vector.tensor_copy(Vb_sb[:, jc * VW: jc * VW + F_OUT], ps_v[:, :])
            vcol = v_sb[:, 2 * jc: 2 * jc + 1]
            v2col = v_sb[:, 2 * jc + 1: 2 * jc + 2]
            nc.vector.tensor_scalar(V1_sb[:, jc * VW:(jc + 1) * VW],
                                    Vb_sb[:, jc * VW:(jc + 1) * VW], vcol, None,
                                    op0=ALU.mult)
            nc.vector.tensor_scalar(V2_sb[:, jc * VW:(jc + 1) * VW],
                                    Vb_sb[:, jc * VW:(jc + 1) * VW], v2col, None,
                                    op0=ALU.mult)
        pre_ctx.close()

        # global coverage of the two accumulator families
        gLO0 = LO[0]            # ACC2 covered on [0, gLO0)
        gLO31 = LO[NCH - 1]     # ACC1 covered on [gLO31, N)

        # ---------------- j-loop over i-quarters ----------------
        for q in range(NQ):
            qb, qe = q * IW, (q + 1) * IW
            half = q // 2
            psA_ctx = ExitStack()
            psA = psA_ctx.enter_context(
                tc.tile_pool(name=f"psA{q}", bufs=1, space="PSUM"))
            a1 = [psA.tile([VW, 512], F32, tag=f"a1_{k}", name=f"a1_{q}_{k}")
                  for k in range(2)]
            a2 = [psA.tile([VW, 512], F32, tag=f"a2_{k}", name=f"a2_{q}_{k}")
                  for k in range(2)]
            # per-bank last-touch chunk for ACC2 / ACC1
            acc2_last = [max((c for c in range(NCH) if LO[c] > qb + 512 * k),
                             default=None) for k in range(2)]
            acc1_piece = [[c for c in range(NCH)
                           if max(LO[c], qb) < qb + 512 * (k + 1)]
                          for k in range(2)]
            # PSUM start=True resets the whole bank's has_written state, so a
            # partial-width start would wipe earlier columns. Zero banks whose
            # first write is partial; fully-covered ACC2 banks start at chunk 0.
            a2_full = [min(LO[0], qe) >= qb + 512 * (k + 1) for k in range(2)]
            for k in range(2):
                if acc1_piece[k]:
                    nc.tensor.matmul(a1[k][:, :], zst_sb[:, :], zmov_sb[:, :],
                                     start=True, stop=False)
                if acc2_last[k] is not None and not a2_full[k]:
                    nc.tensor.matmul(a2[k][:, :], zst_sb[:, :], zmov_sb[:, :],
                                     start=True, stop=False)
            for jc in range(NCH):
                # mask prefetch schedule
                if q == 0:
                    for mj in (2 * jc + 8, 2 * jc + 9):
                        if mj < NCH:
                            issue_mask(0, mj)
                if q == 1:
                    for mj in (2 * jc, 2 * jc + 1):
                        if mj < NCH:
                            issue_mask(1, mj)
                m8t = m_tiles[(half, jc)]
                mbase = half * H2
                lo, hi = LO[jc], HI[jc]
                # band elementwise (clipped to quarter)
                b0, b1 = max(lo, qb), min(hi, qe)
                qb_t = None
                if b1 > b0:
                    bw = b1 - b0
                    q_t = band_pool.tile([128, BWMAX], BF16, tag="qt",
                                         name=f"qt_{q}_{jc}")
                    vcol = v_sb[:, 2 * jc: 2 * jc + 1]
                    v2col = v_sb[:, 2 * jc + 1: 2 * jc + 2]
                    nc.vector.tensor_scalar(q_t[:, :bw], W_bb[:, b0:b1],
                                            v2col, vcol, op0=ALU.mult, op1=ALU.max)
                    qb_t = band_pool.tile([128, BWMAX], BF16, tag="qbt",
                                          name=f"qbt_{q}_{jc}")
                    nc.vector.tensor_tensor(qb_t[:, :bw], q_t[:, :bw],
                                            m8t[:, b0 - mbase:b1 - mbase],
                                            op=ALU.mult)
                # matmuls per 512-bank
                for k in range(2):
                    bs, be = qb + 512 * k, qb + 512 * (k + 1)
                    # ACC2 piece [bs, min(be, lo))
                    e2 = min(be, lo)
                    if e2 > bs:
                        nc.tensor.matmul(
                            a2[k][:, 0:e2 - bs],
                            V2_sb[:, jc * VW:(jc + 1) * VW],
                            m8t[:, bs - mbase:e2 - mbase],
                            start=(a2_full[k] and jc == 0),
                            stop=(jc == acc2_last[k]))
                    # ACC1 pieces [max(bs, lo), be), cut at hi and cov1
                    p0 = max(bs, lo)
                    if p0 < be:
                        cuts = {p0, be}
                        if p0 < hi < be:
                            cuts.add(hi)
                        cuts = sorted(cuts)
                        pieces = list(zip(cuts[:-1], cuts[1:]))
                        last_chunk = (jc == acc1_piece[k][-1]) if acc1_piece[k] else False
                        for pidx, (x0, x1) in enumerate(pieces):
                            in_band = x0 < hi
                            stat = Vb_sb if in_band else V1_sb
                            if in_band:
                                mov = qb_t[:, x0 - b0:x1 - b0]
                            else:
                                mov = m8t[:, x0 - mbase:x1 - mbase]
                            nc.tensor.matmul(
                                a1[k][:, x0 - bs:x1 - bs],
                                stat[:, jc * VW:(jc + 1) * VW],
                                mov,
                                start=False,
                                stop=(last_chunk and pidx == len(pieces) - 1))

            # -------- tail for this quarter --------
            # combine C = ACC1 + W*ACC2 segment-wise into fp32 SBUF
            C_sb = tail_pool.tile([VW, IW], F32, tag="C", name=f"C_{q}")
            for k in range(2):
                bs, be = qb + 512 * k, qb + 512 * (k + 1)
                cuts = {bs, be}
                for g in (gLO31, gLO0):
                    if bs < g < be:
                        cuts.add(g)
                cuts = sorted(cuts)
                for x0, x1 in zip(cuts[:-1], cuts[1:]):
                    lr = slice(x0 - bs, x1 - bs)
                    cr = slice(x0 - qb, x1 - qb)
                    if x1 <= gLO31:       # ACC2 only
                        nc.vector.tensor_tensor(C_sb[:, cr], W_bb[0:VW, x0:x1],
                                                a2[k][:, lr], op=ALU.mult)
                    elif x0 >= gLO0:      # ACC1 only
                        nc.scalar.copy(C_sb[:, cr], a1[k][:, lr])
                    else:                 # both
                        tmp = tail_pool.tile([VW, 512], F32, tag="tmp",
                                             name=f"tmp_{q}_{k}")
                        nc.vector.scalar_tensor_tensor(
                            tmp[:, lr], W_bb[0:VW, x0:x1], 1.0, a2[k][:, lr],
                            op0=ALU.mult, op1=ALU.mult)
                        nc.vector.tensor_tensor(C_sb[:, cr], tmp[:, lr],
                                                a1[k][:, lr], op=ALU.add)
            psA_ctx.close()
            # transpose to [i, o] in packs of 4 per PSUM bank, batch the
            # reciprocal across the pack, normalize on ACT+DVE alternately
            pst_ctx = ExitStack()
            psT = pst_ctx.enter_context(
                tc.tile_pool(name=f"psT{q}", bufs=2, space="PSUM"))
            for pk in range(2):
                ps_t = psT.tile([128, 4 * VW], F32, tag="pst", name=f"pst_{q}_{pk}")
                for m in range(4):
                    icq = 4 * pk + m
                    nc.tensor.transpose(ps_t[:, m * VW:(m + 1) * VW],
                                        C_sb[:, icq * 128:(icq + 1) * 128],
                                        eye65_sb[:, :])
                rec4 = sm_pool.tile([128, 4], F32, tag="rec", name=f"rec_{q}_{pk}")
                nc.vector.reciprocal_approx_fast(rec4[:, :], ps_t[:, F_OUT::VW])
                for m in range(4):
                    ic = q * (IW // 128) + 4 * pk + m
                    o_t = sm_pool.tile([128, F_OUT], F32, tag=f"ot{m % 2}",
                                       name=f"ot_{q}_{pk}_{m}")
                    if m % 2 == 0:
                        nc.scalar.activation(o_t[:, :], ps_t[:, m * VW:m * VW + F_OUT],
                                             AF.Copy, scale=rec4[:, m:m + 1])
                    else:
                        nc.vector.tensor_scalar(o_t[:, :],
                                                ps_t[:, m * VW:m * VW + F_OUT],
                                                rec4[:, m:m + 1], None, op0=ALU.mult)
                    nc.gpsimd.dma_start(out_nat[ic * 128:(ic + 1) * 128, :], o_t[:, :])
            pst_ctx.close()
    nc.compile()
    return nc


_CACHE = {}


def _get_nc(LO, HI, add_b):
    key = (LO, HI, add_b)
    if key not in _CACHE:
        _CACHE[key] = build_program(LO, HI, add_b)
    return _CACHE[key]


def _split_hilo(x):
    hi = x.astype(ml_dtypes.bfloat16)
    lo = (x - hi.astype(np.float32)).astype(ml_dtypes.bfloat16)
    return hi, lo


def _prep(h, adj, w, a_src, a_dst, b):
    h = np.asarray(h, dtype=np.float32)
    adj = np.asarray(adj)
    w = np.asarray(w, dtype=np.float32)
    a_src = np.asarray(a_src, dtype=np.float32)
    a_dst = np.asarray(a_dst, dtype=np.float32)
    b = np.asarray(b, dtype=np.float32)

    perms, LO, HI = _compute_layout(h, w, a_src, a_dst)
    add_b = bool(np.any(b != 0.0))

    eye3 = np.eye(3, dtype=np.float32)
    eye65 = np.eye(VW, dtype=np.float32)
    ones1 = np.ones((1, 128), dtype=np.float32).astype(ml_dtypes.bfloat16)
    b_row = b[None, :].astype(ml_dtypes.bfloat16)
    hT = np.ascontiguousarray(h.T)  # fp32 [F_IN, N]

    in_maps = []
    for c in range(N_HEAD):
        pi, pj = perms[c]
        wa_s = (w[c] @ a_src[c])[:, 0]
        wa_d = (w[c] @ a_dst[c])[:, 0]
        # wa6 cols: 0=d_hi 1=d5_hi 2=d_lo 3=d5_lo 4=s_hi 5=s_lo
        dh, dl = _split_hilo(wa_d)
        d5h, d5l = _split_hilo(NEG * wa_d)
        sh, sl_ = _split_hilo(wa_s)
        wa6 = np.stack([x.astype(np.float32) for x in
                        (dh, d5h, dl, d5l, sh, sl_)], axis=1)
        in_maps.append({
            "hTi": np.ascontiguousarray(hT[:, pi]).astype(ml_dtypes.bfloat16),
            "hTj": np.ascontiguousarray(hT[:, pj]).astype(ml_dtypes.bfloat16),
            "wa6": np.ascontiguousarray(wa6).astype(ml_dtypes.bfloat16),
            "w_bf": np.ascontiguousarray(w[c]).astype(ml_dtypes.bfloat16),
            "eye3": eye3,
            "eye65": eye65,
            "ones1": ones1,
            "b_row": b_row,
            "mask8": np.ascontiguousarray(
                adj.T[np.ix_(pj, pi)]).astype(ml_dtypes.float8_e4m3),
        })
    return in_maps, perms, LO, HI, add_b


def _run(in_maps, LO, HI, add_b, trace=False, **kwargs):
    nc = _get_nc(LO, HI, add_b)
    return run_bass_kernel_spmd(nc, in_maps, list(range(N_HEAD)), trace=trace,
                                **kwargs)


def kernel(h, adj, w, a_src, a_dst, b):
    in_maps, perms, LO, HI, add_b = _prep(h, adj, w, a_src, a_dst, b)
    res = _run(in_maps, LO, HI, add_b)
    out = np.empty((N_HEAD, N, F_OUT), np.float32)
    for c in range(N_HEAD):
        pi, _ = perms[c]
        out[c][pi] = res.results[c]["out_nat"]
    return out


# revision 3
# speedup vs baseline: 1.0909x; 1.0909x over previous
"""Trainium2 Bass kernel for nn_MultiHeadGraphAttention (v3: sorted staircase).

One head per core. Host sorts rows i by s_i and columns j by d_j (pure
layout permutations, applied to the mask / h uploads and undone on the
output). With both axes sorted, sign(s_i + d_j) forms a monotone staircase:
per j-chunk c there are compile-time column boundaries LO_c >= HI_c' ...
    cols [0, LO_c)      : t < 0 for every j in the chunk  (branch 2)
    cols [LO_c, HI_c)   : mixed "band" (~10% of the matrix)
    cols [HI_c, N)      : t >= 0 for every j               (branch 1)

Factoring u_i = exp(s_i) out of every branch (it cancels in the softmax):
    P = u_i * M * max(v_j, w_i * v2_j),  v = exp(d), v2 = exp(0.2 d),
                                         w_i = exp(-0.8 s_i)
    pure-1: P/u = v_j * M        -> PE matmul (V*v  stationary, fp8 mask moving)
    pure-2: P/u = w_i * v2_j * M -> PE matmul (V*v2 stationary), w_i applied
                                    per-column at combine time
    band:   Q = M * max(v_j, w_i v2_j) -> tiny elementwise + PE matmul
so the mask multiply + exp work for 90% of the matrix is done BY THE PE
(fp8 moving operand measured exact and full speed), DVE only touches the
band. out = (ACC1 + w*ACC2)[0:64] / (ACC1 + w*ACC2)[64] column-wise; the
ones-column of each stationary supplies the denominators; u_i cancels.

PSUM holds ACC1+ACC2 for a 1024-column quarter (8 banks); the i-range is
processed in 4 quarters, tail (combine -> transpose -> normalize -> DMA)
of quarter q overlapping the j-loop of quarter q+1.
"""
import sys

if "/opt/trn_rl_repo" not in sys.path:
    sys.path.insert(0, "/opt/trn_rl_repo")

from contextlib import ExitStack

import ml_dtypes
import numpy as np

import concourse.bass as bass
import concourse.bacc as bacc
import concourse.tile as tile
from concourse import mybir
from concourse.bass_utils import run_bass_kernel_spmd

F32 = mybir.dt.float32
BF16 = mybir.dt.bfloat16
FP8 = mybir.dt.float8e4
AF = mybir.ActivationFunctionType
ALU = mybir.AluOpType

N = 4096
F_IN = 256
N_HEAD = 8
F_OUT = 64
NEG = 0.2
NCH = N // 128        # 32 j-chunks
FC = F_IN // 128      # 2 f-chunks
VW = F_OUT + 1        # 65: V columns + ones column
IW = 512              # i-slab width (eighths)
NQ = N // IW
BWMAX = 640           # max band width per chunk (assert against actual)


def _compute_layout(h, w, a_src, a_dst):
    """Per-head sort permutations and union staircase boundaries."""
    h64 = np.asarray(h, np.float64)
    perms = []
    los = np.zeros((N_HEAD, NCH), np.int64)
    his = np.zeros((N_HEAD, NCH), np.int64)
    for c in range(N_HEAD):
        wa_s = (np.asarray(w[c], np.float64) @ np.asarray(a_src[c], np.float64))[:, 0]
        wa_d = (np.asarray(w[c], np.float64) @ np.asarray(a_dst[c], np.float64))[:, 0]
        s = h64 @ wa_s
        d = h64 @ wa_d
        pi = np.argsort(s, kind="stable")
        pj = np.argsort(d, kind="stable")
        ss = s[pi]
        ds = d[pj]
        for jc in range(NCH):
            dmax = ds[jc * 128 + 127]
            dmin = ds[jc * 128]
            los[c, jc] = np.searchsorted(ss, -dmax, side="left")
            his[c, jc] = np.searchsorted(ss, -dmin, side="left")
        perms.append((pi, pj))
    LO = los.min(axis=0)
    HI = his.max(axis=0)
    assert np.all(np.diff(LO) <= 0) and np.all(np.diff(HI) <= 0)
    assert np.all(HI - LO <= BWMAX), f"band too wide: {int((HI-LO).max())}"
    return perms, tuple(int(x) for x in LO), tuple(int(x) for x in HI)


def build_program(LO, HI, add_b):
    nc = bacc.Bacc("TRN2", target_bir_lowering=False, debug=False)
    hTi = nc.dram_tensor("hTi", [F_IN, N], BF16, kind="ExternalInput").ap()
    hTj = nc.dram_tensor("hTj", [F_IN, N], BF16, kind="ExternalInput").ap()
    wa6 = nc.dram_tensor("wa6", [F_IN, 6], BF16, kind="ExternalInput").ap()
    w_bf = nc.dram_tensor("w_bf", [F_IN, F_OUT], BF16, kind="ExternalInput").ap()
    eye3 = nc.dram_tensor("eye3", [3, 3], F32, kind="ExternalInput").ap()
    eye65 = nc.dram_tensor("eye65", [VW, VW], F32, kind="ExternalInput").ap()
    ones1 = nc.dram_tensor("ones1", [1, 128], BF16, kind="ExternalInput").ap()
    b_row = nc.dram_tensor("b_row", [1, F_OUT], BF16, kind="ExternalInput").ap()
    mask8 = nc.dram_tensor("mask8", [N, N], FP8, kind="ExternalInput").ap()
    out_nat = nc.dram_tensor("out_nat", [N, F_OUT], F32, kind="ExternalOutput").ap()
    s_dram = nc.dram_tensor("s_scratch", [N], F32).ap()

    with tile.TileContext(nc) as tc, ExitStack() as ctx:
        const_pool = ctx.enter_context(tc.tile_pool(name="const", bufs=1))
        mask_pool = ctx.enter_context(tc.tile_pool(name="mask", bufs=40))
        tail_pool = ctx.enter_context(tc.tile_pool(name="tail", bufs=2))
        band_pool = ctx.enter_context(tc.tile_pool(name="band", bufs=3))
        sm_pool = ctx.enter_context(tc.tile_pool(name="sm", bufs=4))
        pre_ctx = ExitStack()
        psw_pool = pre_ctx.enter_context(tc.tile_pool(name="psw", bufs=2, space="PSUM"))
        pre_pool = pre_ctx.enter_context(tc.tile_pool(name="pre", bufs=1))

        # ---------------- input loads ----------------
        hTi_sb = pre_pool.tile([128, FC * N], BF16, tag="hTi")
        hTj_sb = pre_pool.tile([128, FC * N], BF16, tag="hTj")
        H2 = N // 2
        for half in range(2):
            for fc in range(FC):
                nc.sync.dma_start(
                    hTi_sb[:, fc * N + half * H2: fc * N + (half + 1) * H2],
                    hTi[fc * 128:(fc + 1) * 128, half * H2:(half + 1) * H2])
        for half in range(2):
            for fc in range(FC):
                nc.sync.dma_start(
                    hTj_sb[:, fc * N + half * H2: fc * N + (half + 1) * H2],
                    hTj[fc * 128:(fc + 1) * 128, half * H2:(half + 1) * H2])
        wa_sb = const_pool.tile([128, FC * 6], BF16, tag="wa")
        for fc in range(FC):
            nc.sync.dma_start(wa_sb[:, fc * 6:(fc + 1) * 6],
                              wa6[fc * 128:(fc + 1) * 128, :])
        w_sb = const_pool.tile([128, FC * F_OUT], BF16, tag="w")
        for fc in range(FC):
            nc.sync.dma_start(w_sb[:, fc * F_OUT:(fc + 1) * F_OUT],
                              w_bf[fc * 128:(fc + 1) * 128, :])
        eye3_sb = const_pool.tile([3, 3], F32, tag="eye3")
        nc.sync.dma_start(eye3_sb[:, :], eye3[:, :])
        eye65_sb = const_pool.tile([VW, VW], F32, tag="eye65")
        nc.sync.dma_start(eye65_sb[:, :], eye65[:, :])
        ones1_sb = const_pool.tile([1, 128], BF16, tag="ones1")
        nc.sync.dma_start(ones1_sb[:, :], ones1[:, :])
        brow_sb = const_pool.tile([1, F_OUT], BF16, tag="brow")
        nc.sync.dma_start(brow_sb[:, :], b_row[:, :])
        zst_sb = const_pool.tile([1, VW], BF16, tag="zst")
        nc.gpsimd.memset(zst_sb[:, :], 0.0)
        zmov_sb = const_pool.tile([1, 512], BF16, tag="zmov")
        nc.gpsimd.memset(zmov_sb[:, :], 0.0)

        # mask tiles: per (half, chunk) [128, 2048] fp8
        m_tiles = {}

        def issue_mask(half, jc):
            t = mask_pool.tile([128, N // 2], FP8, tag="m8", name=f"m8_{half}_{jc}")
            eng = nc.sync if jc % 2 == 0 else nc.scalar
            eng.dma_start(t[:, :], mask8[jc * 128:(jc + 1) * 128,
                                         half * H2:(half + 1) * H2])
            m_tiles[(half, jc)] = t

        for jc in range(8):
            issue_mask(0, jc)

        # ---------------- s row (from hTi) ----------------
        srow_sb = pre_pool.tile([1, N], F32, tag="srow")
        for sl in range(8):
            ps_s = psw_pool.tile([1, 512], F32, tag="pss")
            combos = [(fc, hl) for fc in range(FC) for hl in range(2)]
            for ci, (fc, hl) in enumerate(combos):
                nc.tensor.matmul(ps_s[:, :],
                                 wa_sb[:, fc * 6 + 4 + hl: fc * 6 + 5 + hl],
                                 hTi_sb[:, fc * N + sl * 512: fc * N + (sl + 1) * 512],
                                 start=(ci == 0), stop=(ci == len(combos) - 1))
            nc.vector.tensor_copy(srow_sb[0:1, sl * 512:(sl + 1) * 512], ps_s[:, :])
        # broadcast s to partitions, then W = exp(-0.8 s) table
        S_b = pre_pool.tile([128, N], F32, tag="Sb")
        for half in range(2):
            hs = slice(half * H2, (half + 1) * H2)
            nc.sync.dma_start(s_dram[hs], srow_sb[0:1, hs])
            nc.sync.dma_start(S_b[:, hs], s_dram[None, hs].broadcast_to((128, H2)))
        W_bb = const_pool.tile([128, N], BF16, tag="Wbb")
        nc.scalar.activation(W_bb[:, :], S_b[:, :], AF.Exp, scale=-0.8)

        # ---------------- d rows (from hTj) + transposes ----------------
        ddT_sb = pre_pool.tile([2, N], F32, tag="ddT")
        for sl in range(8):
            ps_d = psw_pool.tile([2, 512], F32, tag="psd")
            combos = [(fc, hl) for fc in range(FC) for hl in range(2)]
            for ci, (fc, hl) in enumerate(combos):
                nc.tensor.matmul(ps_d[:, :],
                                 wa_sb[:, fc * 6 + 2 * hl: fc * 6 + 2 * hl + 2],
                                 hTj_sb[:, fc * N + sl * 512: fc * N + (sl + 1) * 512],
                                 start=(ci == 0), stop=(ci == len(combos) - 1))
            nc.vector.tensor_copy(ddT_sb[0:2, sl * 512:(sl + 1) * 512], ps_d[:, :])
        d_sb = const_pool.tile([128, 2 * NCH], F32, tag="d")
        for jc in range(NCH):
            ps_td = psw_pool.tile([128, 2], F32, tag="pstd")
            nc.tensor.transpose(ps_td[:, :], ddT_sb[0:2, jc * 128:(jc + 1) * 128],
                                eye3_sb[0:2, 0:2])
            nc.vector.tensor_copy(d_sb[:, 2 * jc: 2 * jc + 2], ps_td[:, :])
        v_sb = const_pool.tile([128, 2 * NCH], F32, tag="v")
        nc.scalar.activation(v_sb[:, :], d_sb[:, :], AF.Exp)

        # ---------------- V (h_prime + b) and scaled stationaries ----------------
        Vb_sb = const_pool.tile([128, NCH * VW], BF16, tag="Vb")
        V1_sb = const_pool.tile([128, NCH * VW], BF16, tag="V1")
        V2_sb = const_pool.tile([128, NCH * VW], BF16, tag="V2")
        nc.vector.memset(Vb_sb[:, :], 1.0)
        for jc in range(NCH):
            ps_v = psw_pool.tile([128, F_OUT], F32, tag="psv")
            for fc in range(FC):
                nc.tensor.matmul(
                    ps_v[:, :],
                    hTj_sb[:, fc * N + jc * 128: fc * N + (jc + 1) * 128],
                    w_sb[:, fc * F_OUT:(fc + 1) * F_OUT],
                    start=(fc == 0), stop=(not add_b and fc == FC - 1))
            if add_b:
                nc.tensor.matmul(ps_v[:, :], ones1_sb[:, :], brow_sb[:, :],
                                 start=False, stop=True)
            if jc % 2 == 0:
                nc.scalar.copy(Vb_sb[:, jc * VW: jc * VW + F_OUT], ps_v[:, :])
            else:
                nc.vector.tensor_copy(Vb_sb[:, jc * VW: jc * VW + F_OUT], ps_v[:, :])
            vcol = v_sb[:, 2 * jc: 2 * jc + 1]
            v2col = v_sb[:, 2 * jc + 1: 2 * jc + 2]
            nc.vector.tensor_scalar(V1_sb[:, jc * VW:(jc + 1) * VW],
                                    Vb_sb[:, jc * VW:(jc + 1) * VW], vcol, None,
                                    op0=ALU.mult)
            nc.vector.tensor_scalar(V2_sb[:, jc * VW:(jc + 1) * VW],
                                    Vb_sb[:, jc * VW:(jc + 1) * VW], v2col, None,
                                    op0=ALU.mult)
        pre_ctx.close()

        # global coverage of the two accumulator families
        gLO0 = LO[0]            # ACC2 covered on [0, gLO0)
        gLO31 = LO[NCH - 1]     # ACC1 covered on [gLO31, N)

        # ---------------- j-loop over i-eighths, tails deferred ----------
        # Persistent ring pools: slab q uses buf q%2; WAR deps via the pool
        # ring replace explicit scoping, so slab q+1's matmuls never sit
        # behind slab q's tail in the PE queue.
        psacc = ctx.enter_context(tc.tile_pool(name="psacc", bufs=2, space="PSUM"))
        psT = ctx.enter_context(tc.tile_pool(name="psT", bufs=2, space="PSUM"))
        gLO0 = LO[0]
        gLO31 = LO[NCH - 1]

        def emit_tail(q, a1, a2):
            qb, qe = q * IW, (q + 1) * IW
            C_sb = tail_pool.tile([VW, IW], F32, tag="C", name=f"C_{q}")
            cuts = {qb, qe}
            for g in (gLO31, gLO0):
                if qb < g < qe:
                    cuts.add(g)
            cuts = sorted(cuts)
            for x0, x1 in zip(cuts[:-1], cuts[1:]):
                lr = slice(x0 - qb, x1 - qb)
                if x1 <= gLO31:       # ACC2 only
                    nc.vector.tensor_tensor(C_sb[:, lr], W_bb[0:VW, x0:x1],
                                            a2[:, lr], op=ALU.mult)
                elif x0 >= gLO0:      # ACC1 only
                    nc.scalar.copy(C_sb[:, lr], a1[:, lr])
                else:                 # both
                    tmp = tail_pool.tile([VW, 512], F32, tag="tmp",
                                         name=f"tmp_{q}")
                    nc.vector.scalar_tensor_tensor(
                        tmp[:, lr], W_bb[0:VW, x0:x1], 1.0, a2[:, lr],
                        op0=ALU.mult, op1=ALU.mult)
                    nc.vector.tensor_tensor(C_sb[:, lr], tmp[:, lr],
                                            a1[:, lr], op=ALU.add)
            ps_t = psT.tile([128, 4 * VW], F32, tag="pst", name=f"pst_{q}")
            for m in range(4):
                nc.tensor.transpose(ps_t[:, m * VW:(m + 1) * VW],
                                    C_sb[:, m * 128:(m + 1) * 128],
                                    eye65_sb[:, :])
            rec4 = sm_pool.tile([128, 4], F32, tag="rec", name=f"rec_{q}")
            nc.vector.reciprocal_approx_fast(rec4[:, :], ps_t[:, F_OUT::VW])
            for m in range(4):
                ic = q * 4 + m
                o_t = sm_pool.tile([128, F_OUT], F32, tag=f"ot{m % 2}",
                                   name=f"ot_{q}_{m}")
                if m % 2 == 0:
                    nc.scalar.activation(o_t[:, :], ps_t[:, m * VW:m * VW + F_OUT],
                                         AF.Copy, scale=rec4[:, m:m + 1])
                else:
                    nc.vector.tensor_scalar(o_t[:, :],
                                            ps_t[:, m * VW:m * VW + F_OUT],
                                            rec4[:, m:m + 1], None, op0=ALU.mult)
                nc.gpsimd.dma_start(out_nat[ic * 128:(ic + 1) * 128, :], o_t[:, :])

        pending = None
        for q in range(NQ):
            qb, qe = q * IW, (q + 1) * IW
            half = q // 4
            a1 = psacc.tile([VW, 512], F32, tag="a1", name=f"a1_{q}")
            a2 = psacc.tile([VW, 512], F32, tag="a2", name=f"a2_{q}")
            acc2_last = max((c for c in range(NCH) if LO[c] > qb), default=None)
            acc1_chunks = [c for c in range(NCH) if max(LO[c], qb) < qe]
            a2_full = min(LO[0], qe) >= qe
            nc.tensor.matmul(a1[:, :], zst_sb[:, :], zmov_sb[:, :],
                             start=True, stop=(not acc1_chunks))
            if not a2_full:
                nc.tensor.matmul(a2[:, :], zst_sb[:, :], zmov_sb[:, :],
                                 start=True, stop=(acc2_last is None))
            for jc in range(NCH):
                if q == 0:
                    for mj in (2 * jc + 8, 2 * jc + 9):
                        if mj < NCH:
                            issue_mask(0, mj)
                if q == 2:
                    for mj in (2 * jc, 2 * jc + 1):
                        if mj < NCH:
                            issue_mask(1, mj)
                m8t = m_tiles[(half, jc)]
                mbase = half * H2
                lo, hi = LO[jc], HI[jc]
                b0, b1 = max(lo, qb), min(hi, qe)
                qb_t = None
                if b1 > b0:
                    bw = b1 - b0
                    q_t = band_pool.tile([128, BWMAX], BF16, tag="qt",
                                         name=f"qt_{q}_{jc}")
                    vcol = v_sb[:, 2 * jc: 2 * jc + 1]
                    v2col = v_sb[:, 2 * jc + 1: 2 * jc + 2]
                    nc.vector.tensor_scalar(q_t[:, :bw], W_bb[:, b0:b1],
                                            v2col, vcol, op0=ALU.mult, op1=ALU.max)
                    qb_t = band_pool.tile([128, BWMAX], BF16, tag="qbt",
                                          name=f"qbt_{q}_{jc}")
                    nc.vector.tensor_tensor(qb_t[:, :bw], q_t[:, :bw],
                                            m8t[:, b0 - mbase:b1 - mbase],
                                            op=ALU.mult)
                e2 = min(qe, lo)
                if e2 > qb:
                    nc.tensor.matmul(
                        a2[:, 0:e2 - qb],
                        V2_sb[:, jc * VW:(jc + 1) * VW],
                        m8t[:, qb - mbase:e2 - mbase],
                        start=(a2_full and jc == 0),
                        stop=(jc == acc2_last))
                p0 = max(qb, lo)
                if p0 < qe:
                    cuts = {p0, qe}
                    if p0 < hi < qe:
                        cuts.add(hi)
                    cuts = sorted(cuts)
                    pieces = list(zip(cuts[:-1], cuts[1:]))
                    last_chunk = (jc == acc1_chunks[-1]) if acc1_chunks else False
                    for pidx, (x0, x1) in enumerate(pieces):
                        in_band = x0 < hi
                        stat = Vb_sb if in_band else V1_sb
                        if in_band:
                            mov = qb_t[:, x0 - b0:x1 - b0]
                        else:
                            mov = m8t[:, x0 - mbase:x1 - mbase]
                        nc.tensor.matmul(
                            a1[:, x0 - qb:x1 - qb],
                            stat[:, jc * VW:(jc + 1) * VW],
                            mov,
                            start=False,
                            stop=(last_chunk and pidx == len(pieces) - 1))
            if pending is not None:
                emit_tail(*pending)
            pending = (q, a1, a2)
        emit_tail(*pending)
    nc.compile()
    return nc


_CACHE = {}


def _get_nc(LO, HI, add_b):
    key = (LO, HI, add_b)
    if key not in _CACHE:
        _CACHE[key] = build_program(LO, HI, add_b)
    return _CACHE[key]


def _split_hilo(x):
    hi = x.astype(ml_dtypes.bfloat16)
    lo = (x - hi.astype(np.float32)).astype(ml_dtypes.bfloat16)
    return hi, lo


def _prep(h, adj, w, a_src, a_dst, b):
    h = np.asarray(h, dtype=np.float32)
    adj = np.asarray(adj)
    w = np.asarray(w, dtype=np.float32)
    a_src = np.asarray(a_src, dtype=np.float32)
    a_dst = np.asarray(a_dst, dtype=np.float32)
    b = np.asarray(b, dtype=np.float32)

    perms, LO, HI = _compute_layout(h, w, a_src, a_dst)
    add_b = bool(np.any(b != 0.0))

    eye3 = np.eye(3, dtype=np.float32)
    eye65 = np.eye(VW, dtype=np.float32)
    ones1 = np.ones((1, 128), dtype=np.float32).astype(ml_dtypes.bfloat16)
    b_row = b[None, :].astype(ml_dtypes.bfloat16)
    hT = np.ascontiguousarray(h.T)  # fp32 [F_IN, N]

    in_maps = []
    for c in range(N_HEAD):
        pi, pj = perms[c]
        wa_s = (w[c] @ a_src[c])[:, 0]
        wa_d = (w[c] @ a_dst[c])[:, 0]
        # wa6 cols: 0=d_hi 1=d5_hi 2=d_lo 3=d5_lo 4=s_hi 5=s_lo
        dh, dl = _split_hilo(wa_d)
        d5h, d5l = _split_hilo(NEG * wa_d)
        sh, sl_ = _split_hilo(wa_s)
        wa6 = np.stack([x.astype(np.float32) for x in
                        (dh, d5h, dl, d5l, sh, sl_)], axis=1)
        in_maps.append({
            "hTi": np.ascontiguousarray(hT[:, pi]).astype(ml_dtypes.bfloat16),
            "hTj": np.ascontiguousarray(hT[:, pj]).astype(ml_dtypes.bfloat16),
            "wa6": np.ascontiguousarray(wa6).astype(ml_dtypes.bfloat16),
            "w_bf": np.ascontiguousarray(w[c]).astype(ml_dtypes.bfloat16),
            "eye3": eye3,
            "eye65": eye65,
            "ones1": ones1,
            "b_row": b_row,
            "mask8": np.ascontiguousarray(
                adj.T[np.ix_(pj, pi)]).astype(ml_dtypes.float8_e4m3),
        })
    return in_maps, perms, LO, HI, add_b


def _run(in_maps, LO, HI, add_b, trace=False, **kwargs):
    nc = _get_nc(LO, HI, add_b)
    return run_bass_kernel_spmd(nc, in_maps, list(range(N_HEAD)), trace=trace,
                                **kwargs)


def kernel(h, adj, w, a_src, a_dst, b):
    in_maps, perms, LO, HI, add_b = _prep(h, adj, w, a_src, a_dst, b)
    res = _run(in_maps, LO, HI, add_b)
    out = np.empty((N_HEAD, N, F_OUT), np.float32)
    for c in range(N_HEAD):
        pi, _ = perms[c]
        out[c][pi] = res.results[c]["out_nat"]
    return out


# revision 4
# speedup vs baseline: 1.1532x; 1.0571x over previous
"""Trainium2 Bass kernel for nn_MultiHeadGraphAttention (v3: sorted staircase).

One head per core. Host sorts rows i by s_i and columns j by d_j (pure
layout permutations, applied to the mask / h uploads and undone on the
output). With both axes sorted, sign(s_i + d_j) forms a monotone staircase:
per j-chunk c there are compile-time column boundaries LO_c >= HI_c' ...
    cols [0, LO_c)      : t < 0 for every j in the chunk  (branch 2)
    cols [LO_c, HI_c)   : mixed "band" (~10% of the matrix)
    cols [HI_c, N)      : t >= 0 for every j               (branch 1)

Factoring u_i = exp(s_i) out of every branch (it cancels in the softmax):
    P = u_i * M * max(v_j, w_i * v2_j),  v = exp(d), v2 = exp(0.2 d),
                                         w_i = exp(-0.8 s_i)
    pure-1: P/u = v_j * M        -> PE matmul (V*v  stationary, fp8 mask moving)
    pure-2: P/u = w_i * v2_j * M -> PE matmul (V*v2 stationary), w_i applied
                                    per-column at combine time
    band:   Q = M * max(v_j, w_i v2_j) -> tiny elementwise + PE matmul
so the mask multiply + exp work for 90% of the matrix is done BY THE PE
(fp8 moving operand measured exact and full speed), DVE only touches the
band. out = (ACC1 + w*ACC2)[0:64] / (ACC1 + w*ACC2)[64] column-wise; the
ones-column of each stationary supplies the denominators; u_i cancels.

PSUM holds ACC1+ACC2 for a 1024-column quarter (8 banks); the i-range is
processed in 4 quarters, tail (combine -> transpose -> normalize -> DMA)
of quarter q overlapping the j-loop of quarter q+1.
"""
import sys

if "/opt/trn_rl_repo" not in sys.path:
    sys.path.insert(0, "/opt/trn_rl_repo")

from contextlib import ExitStack

import ml_dtypes
import numpy as np

import concourse.bass as bass
import concourse.bacc as bacc
import concourse.tile as tile
from concourse import mybir
from concourse.bass_utils import run_bass_kernel_spmd

F32 = mybir.dt.float32
BF16 = mybir.dt.bfloat16
FP8 = mybir.dt.float8e4
AF = mybir.ActivationFunctionType
ALU = mybir.AluOpType

N = 4096
F_IN = 256
N_HEAD = 8
F_OUT = 64
NEG = 0.2
NCH = N // 128        # 32 j-chunks
FC = F_IN // 128      # 2 f-chunks
VW = F_OUT + 1        # 65: V columns + ones column
IW = 512              # i-slab width (eighths)
NQ = N // IW
BWMAX = 640           # max band width per chunk (assert against actual)


def _compute_layout(h, w, a_src, a_dst):
    """Per-head sort permutations and union staircase boundaries."""
    h64 = np.asarray(h, np.float64)
    perms = []
    los = np.zeros((N_HEAD, NCH), np.int64)
    his = np.zeros((N_HEAD, NCH), np.int64)
    for c in range(N_HEAD):
        wa_s = (np.asarray(w[c], np.float64) @ np.asarray(a_src[c], np.float64))[:, 0]
        wa_d = (np.asarray(w[c], np.float64) @ np.asarray(a_dst[c], np.float64))[:, 0]
        s = h64 @ wa_s
        d = h64 @ wa_d
        pi = np.argsort(s, kind="stable")
        pj = np.argsort(d, kind="stable")
        ss = s[pi]
        ds = d[pj]
        for jc in range(NCH):
            dmax = ds[jc * 128 + 127]
            dmin = ds[jc * 128]
            los[c, jc] = np.searchsorted(ss, -dmax, side="left")
            his[c, jc] = np.searchsorted(ss, -dmin, side="left")
        perms.append((pi, pj))
    LO = los.min(axis=0)
    HI = his.max(axis=0)
    assert np.all(np.diff(LO) <= 0) and np.all(np.diff(HI) <= 0)
    assert np.all(HI - LO <= BWMAX), f"band too wide: {int((HI-LO).max())}"
    return perms, tuple(int(x) for x in LO), tuple(int(x) for x in HI)


def build_program(LO, HI, add_b):
    nc = bacc.Bacc("TRN2", target_bir_lowering=False, debug=False)
    hTi = nc.dram_tensor("hTi", [F_IN, N], BF16, kind="ExternalInput").ap()
    hTj = nc.dram_tensor("hTj", [F_IN, N], BF16, kind="ExternalInput").ap()
    wa6 = nc.dram_tensor("wa6", [F_IN, 6], BF16, kind="ExternalInput").ap()
    w_bf = nc.dram_tensor("w_bf", [F_IN, F_OUT], BF16, kind="ExternalInput").ap()
    eye3 = nc.dram_tensor("eye3", [3, 3], F32, kind="ExternalInput").ap()
    eye65 = nc.dram_tensor("eye65", [VW, VW], F32, kind="ExternalInput").ap()
    ones1 = nc.dram_tensor("ones1", [1, 128], BF16, kind="ExternalInput").ap()
    b_row = nc.dram_tensor("b_row", [1, F_OUT], BF16, kind="ExternalInput").ap()
    mask8 = nc.dram_tensor("mask8", [N, N], FP8, kind="ExternalInput").ap()
    out_nat = nc.dram_tensor("out_nat", [N, F_OUT], F32, kind="ExternalOutput").ap()
    s_dram = nc.dram_tensor("s_scratch", [N], F32).ap()

    with tile.TileContext(nc) as tc, ExitStack() as ctx:
        const_pool = ctx.enter_context(tc.tile_pool(name="const", bufs=1))
        mask_pool = ctx.enter_context(tc.tile_pool(name="mask", bufs=40))
        tail_pool = ctx.enter_context(tc.tile_pool(name="tail", bufs=2))
        band_pool = ctx.enter_context(tc.tile_pool(name="band", bufs=3))
        sm_pool = ctx.enter_context(tc.tile_pool(name="sm", bufs=4))
        pre_ctx = ExitStack()
        psw_pool = pre_ctx.enter_context(tc.tile_pool(name="psw", bufs=2, space="PSUM"))
        pre_pool = pre_ctx.enter_context(tc.tile_pool(name="pre", bufs=1))

        # ---------------- input loads (small stationaries first) ----------------
        hTi_sb = pre_pool.tile([128, FC * N], BF16, tag="hTi")
        hTj_sb = pre_pool.tile([128, FC * N], BF16, tag="hTj")
        H2 = N // 2
        wa_sb = const_pool.tile([128, FC * 6], BF16, tag="wa")
        for fc in range(FC):
            nc.sync.dma_start(wa_sb[:, fc * 6:(fc + 1) * 6],
                              wa6[fc * 128:(fc + 1) * 128, :])
        w_sb = const_pool.tile([128, FC * F_OUT], BF16, tag="w")
        for fc in range(FC):
            nc.sync.dma_start(w_sb[:, fc * F_OUT:(fc + 1) * F_OUT],
                              w_bf[fc * 128:(fc + 1) * 128, :])
        eye3_sb = const_pool.tile([3, 3], F32, tag="eye3")
        nc.sync.dma_start(eye3_sb[:, :], eye3[:, :])
        eye65_sb = const_pool.tile([VW, VW], F32, tag="eye65")
        nc.sync.dma_start(eye65_sb[:, :], eye65[:, :])
        for half in range(2):
            for fc in range(FC):
                nc.sync.dma_start(
                    hTi_sb[:, fc * N + half * H2: fc * N + (half + 1) * H2],
                    hTi[fc * 128:(fc + 1) * 128, half * H2:(half + 1) * H2])
        for half in range(2):
            for fc in range(FC):
                nc.sync.dma_start(
                    hTj_sb[:, fc * N + half * H2: fc * N + (half + 1) * H2],
                    hTj[fc * 128:(fc + 1) * 128, half * H2:(half + 1) * H2])
        ones1_sb = const_pool.tile([1, 128], BF16, tag="ones1")
        nc.sync.dma_start(ones1_sb[:, :], ones1[:, :])
        brow_sb = const_pool.tile([1, F_OUT], BF16, tag="brow")
        nc.sync.dma_start(brow_sb[:, :], b_row[:, :])
        zst_sb = const_pool.tile([1, VW], BF16, tag="zst")
        nc.gpsimd.memset(zst_sb[:, :], 0.0)
        zmov_sb = const_pool.tile([1, 512], BF16, tag="zmov")
        nc.gpsimd.memset(zmov_sb[:, :], 0.0)

        # mask tiles: per (half, chunk) [128, 2048] fp8
        m_tiles = {}

        def issue_mask(half, jc, eng=None):
            t = mask_pool.tile([128, N // 2], FP8, tag="m8", name=f"m8_{half}_{jc}")
            if eng is None:
                eng = nc.sync if jc % 2 == 0 else nc.scalar
            eng.dma_start(t[:, :], mask8[jc * 128:(jc + 1) * 128,
                                         half * H2:(half + 1) * H2])
            m_tiles[(half, jc)] = t

        for jc in range(8):
            issue_mask(0, jc, eng=nc.sync)

        # ---------------- s row (from hTi) ----------------
        srow_sb = pre_pool.tile([1, N], F32, tag="srow")
        for sl in range(8):
            ps_s = psw_pool.tile([1, 512], F32, tag="pss")
            combos = [(fc, 0) for fc in range(FC)]
            for ci, (fc, hl) in enumerate(combos):
                nc.tensor.matmul(ps_s[:, :],
                                 wa_sb[:, fc * 6 + 4 + hl: fc * 6 + 5 + hl],
                                 hTi_sb[:, fc * N + sl * 512: fc * N + (sl + 1) * 512],
                                 start=(ci == 0), stop=(ci == len(combos) - 1))
            nc.vector.tensor_copy(srow_sb[0:1, sl * 512:(sl + 1) * 512], ps_s[:, :])
        # broadcast s to partitions, then W = exp(-0.8 s) table
        S_b = pre_pool.tile([128, N], F32, tag="Sb")
        for half in range(2):
            hs = slice(half * H2, (half + 1) * H2)
            nc.scalar.dma_start(s_dram[hs], srow_sb[0:1, hs])
            nc.scalar.dma_start(S_b[:, hs], s_dram[None, hs].broadcast_to((128, H2)))
        W_bb = const_pool.tile([128, N], BF16, tag="Wbb")
        nc.scalar.activation(W_bb[:, :], S_b[:, :], AF.Exp, scale=-0.8)

        # ---------------- d rows (from hTj) + transposes ----------------
        ddT_sb = pre_pool.tile([2, N], F32, tag="ddT")
        for sl in range(8):
            ps_d = psw_pool.tile([2, 512], F32, tag="psd")
            combos = [(fc, 0) for fc in range(FC)]
            for ci, (fc, hl) in enumerate(combos):
                nc.tensor.matmul(ps_d[:, :],
                                 wa_sb[:, fc * 6 + 2 * hl: fc * 6 + 2 * hl + 2],
                                 hTj_sb[:, fc * N + sl * 512: fc * N + (sl + 1) * 512],
                                 start=(ci == 0), stop=(ci == len(combos) - 1))
            nc.vector.tensor_copy(ddT_sb[0:2, sl * 512:(sl + 1) * 512], ps_d[:, :])
        d_sb = const_pool.tile([128, 2 * NCH], F32, tag="d")
        for jc in range(NCH):
            ps_td = psw_pool.tile([128, 2], F32, tag="pstd")
            nc.tensor.transpose(ps_td[:, :], ddT_sb[0:2, jc * 128:(jc + 1) * 128],
                                eye3_sb[0:2, 0:2])
            nc.vector.tensor_copy(d_sb[:, 2 * jc: 2 * jc + 2], ps_td[:, :])
        v_sb = const_pool.tile([128, 2 * NCH], F32, tag="v")
        nc.scalar.activation(v_sb[:, :], d_sb[:, :], AF.Exp)

        # ---------------- V (h_prime + b) and scaled stationaries ----------------
        Vb_sb = const_pool.tile([128, NCH * VW], BF16, tag="Vb")
        V1_sb = const_pool.tile([128, NCH * VW], BF16, tag="V1")
        V2_sb = const_pool.tile([128, NCH * VW], BF16, tag="V2")
        nc.vector.memset(Vb_sb[:, :], 1.0)
        for jc in range(NCH):
            ps_v = psw_pool.tile([128, F_OUT], F32, tag="psv")
            for fc in range(FC):
                nc.tensor.matmul(
                    ps_v[:, :],
                    hTj_sb[:, fc * N + jc * 128: fc * N + (jc + 1) * 128],
                    w_sb[:, fc * F_OUT:(fc + 1) * F_OUT],
                    start=(fc == 0), stop=(not add_b and fc == FC - 1))
            if add_b:
                nc.tensor.matmul(ps_v[:, :], ones1_sb[:, :], brow_sb[:, :],
                                 start=False, stop=True)
            if jc % 2 == 0:
                nc.scalar.copy(Vb_sb[:, jc * VW: jc * VW + F_OUT], ps_v[:, :])
            else:
                nc.vector.tensor_copy(Vb_sb[:, jc * VW: jc * VW + F_OUT], ps_v[:, :])
            vcol = v_sb[:, 2 * jc: 2 * jc + 1]
            v2col = v_sb[:, 2 * jc + 1: 2 * jc + 2]
            nc.vector.tensor_scalar(V1_sb[:, jc * VW:(jc + 1) * VW],
                                    Vb_sb[:, jc * VW:(jc + 1) * VW], vcol, None,
                                    op0=ALU.mult)
            nc.vector.tensor_scalar(V2_sb[:, jc * VW:(jc + 1) * VW],
                                    Vb_sb[:, jc * VW:(jc + 1) * VW], v2col, None,
                                    op0=ALU.mult)
        pre_ctx.close()

        # global coverage of the two accumulator families
        gLO0 = LO[0]            # ACC2 covered on [0, gLO0)
        gLO31 = LO[NCH - 1]     # ACC1 covered on [gLO31, N)

        # ---------------- j-loop over i-eighths, tails deferred ----------
        # Persistent ring pools: slab q uses buf q%2; WAR deps via the pool
        # ring replace explicit scoping, so slab q+1's matmuls never sit
        # behind slab q's tail in the PE queue.
        psacc = ctx.enter_context(tc.tile_pool(name="psacc", bufs=2, space="PSUM"))
        psT = ctx.enter_context(tc.tile_pool(name="psT", bufs=2, space="PSUM"))
        gLO0 = LO[0]
        gLO31 = LO[NCH - 1]

        def emit_tail(q, a1, a2):
            qb, qe = q * IW, (q + 1) * IW
            C_sb = tail_pool.tile([VW, IW], F32, tag="C", name=f"C_{q}")
            cuts = {qb, qe}
            for g in (gLO31, gLO0):
                if qb < g < qe:
                    cuts.add(g)
            cuts = sorted(cuts)
            for x0, x1 in zip(cuts[:-1], cuts[1:]):
                lr = slice(x0 - qb, x1 - qb)
                if x1 <= gLO31:       # ACC2 only
                    nc.vector.tensor_tensor(C_sb[:, lr], W_bb[0:VW, x0:x1],
                                            a2[:, lr], op=ALU.mult)
                elif x0 >= gLO0:      # ACC1 only
                    nc.scalar.copy(C_sb[:, lr], a1[:, lr])
                else:                 # both
                    tmp = tail_pool.tile([VW, 512], F32, tag="tmp",
                                         name=f"tmp_{q}")
                    nc.vector.scalar_tensor_tensor(
                        tmp[:, lr], W_bb[0:VW, x0:x1], 1.0, a2[:, lr],
                        op0=ALU.mult, op1=ALU.mult)
                    nc.vector.tensor_tensor(C_sb[:, lr], tmp[:, lr],
                                            a1[:, lr], op=ALU.add)
            ps_t = psT.tile([128, 4 * VW], F32, tag="pst", name=f"pst_{q}")
            for m in range(4):
                nc.tensor.transpose(ps_t[:, m * VW:(m + 1) * VW],
                                    C_sb[:, m * 128:(m + 1) * 128],
                                    eye65_sb[:, :])
            rec4 = sm_pool.tile([128, 4], F32, tag="rec", name=f"rec_{q}")
            nc.vector.reciprocal_approx_fast(rec4[:, :], ps_t[:, F_OUT::VW])
            for m in range(4):
                ic = q * 4 + m
                o_t = sm_pool.tile([128, F_OUT], F32, tag=f"ot{m % 2}",
                                   name=f"ot_{q}_{m}")
                if m % 2 == 0:
                    nc.scalar.activation(o_t[:, :], ps_t[:, m * VW:m * VW + F_OUT],
                                         AF.Copy, scale=rec4[:, m:m + 1])
                else:
                    nc.vector.tensor_scalar(o_t[:, :],
                                            ps_t[:, m * VW:m * VW + F_OUT],
                                            rec4[:, m:m + 1], None, op0=ALU.mult)
                nc.gpsimd.dma_start(out_nat[ic * 128:(ic + 1) * 128, :], o_t[:, :])

        pending = None
        for q in range(NQ):
            qb, qe = q * IW, (q + 1) * IW
            half = q // 4
            a1 = psacc.tile([VW, 512], F32, tag="a1", name=f"a1_{q}")
            a2 = psacc.tile([VW, 512], F32, tag="a2", name=f"a2_{q}")
            acc2_last = max((c for c in range(NCH) if LO[c] > qb), default=None)
            acc1_chunks = [c for c in range(NCH) if max(LO[c], qb) < qe]
            a2_full = min(LO[0], qe) >= qe
            nc.tensor.matmul(a1[:, :], zst_sb[:, :], zmov_sb[:, :],
                             start=True, stop=(not acc1_chunks))
            if not a2_full:
                nc.tensor.matmul(a2[:, :], zst_sb[:, :], zmov_sb[:, :],
                                 start=True, stop=(acc2_last is None))
            for jc in range(NCH):
                if q == 0:
                    for mj in (2 * jc + 8, 2 * jc + 9):
                        if mj < NCH:
                            issue_mask(0, mj)
                if q == 2:
                    for mj in (2 * jc, 2 * jc + 1):
                        if mj < NCH:
                            issue_mask(1, mj)
                m8t = m_tiles[(half, jc)]
                mbase = half * H2
                lo, hi = LO[jc], HI[jc]
                b0, b1 = max(lo, qb), min(hi, qe)
                qb_t = None
                if b1 > b0:
                    bw = b1 - b0
                    q_t = band_pool.tile([128, BWMAX], BF16, tag="qt",
                                         name=f"qt_{q}_{jc}")
                    vcol = v_sb[:, 2 * jc: 2 * jc + 1]
                    v2col = v_sb[:, 2 * jc + 1: 2 * jc + 2]
                    nc.vector.tensor_scalar(q_t[:, :bw], W_bb[:, b0:b1],
                                            v2col, vcol, op0=ALU.mult, op1=ALU.max)
                    qb_t = band_pool.tile([128, BWMAX], BF16, tag="qbt",
                                          name=f"qbt_{q}_{jc}")
                    nc.vector.tensor_tensor(qb_t[:, :bw], q_t[:, :bw],
                                            m8t[:, b0 - mbase:b1 - mbase],
                                            op=ALU.mult)
                e2 = min(qe, lo)
                if e2 > qb:
                    nc.tensor.matmul(
                        a2[:, 0:e2 - qb],
                        V2_sb[:, jc * VW:(jc + 1) * VW],
                        m8t[:, qb - mbase:e2 - mbase],
                        start=(a2_full and jc == 0),
                        stop=(jc == acc2_last))
                p0 = max(qb, lo)
                if p0 < qe:
                    cuts = {p0, qe}
                    if p0 < hi < qe:
                        cuts.add(hi)
                    cuts = sorted(cuts)
                    pieces = list(zip(cuts[:-1], cuts[1:]))
                    last_chunk = (jc == acc1_chunks[-1]) if acc1_chunks else False
                    for pidx, (x0, x1) in enumerate(pieces):
                        in_band = x0 < hi
                        stat = Vb_sb if in_band else V1_sb
                        if in_band:
                            mov = qb_t[:, x0 - b0:x1 - b0]
                        else:
                            mov = m8t[:, x0 - mbase:x1 - mbase]
                        nc.tensor.matmul(
                            a1[:, x0 - qb:x1 - qb],
                            stat[:, jc * VW:(jc + 1) * VW],
                            mov,
                            start=False,
                            stop=(last_chunk and pidx == len(pieces) - 1))
            if pending is not None:
                emit_tail(*pending)
            pending = (q, a1, a2)
        emit_tail(*pending)
    nc.compile()
    return nc


_CACHE = {}


def _get_nc(LO, HI, add_b):
    key = (LO, HI, add_b)
    if key not in _CACHE:
        _CACHE[key] = build_program(LO, HI, add_b)
    return _CACHE[key]


def _split_hilo(x):
    hi = x.astype(ml_dtypes.bfloat16)
    lo = (x - hi.astype(np.float32)).astype(ml_dtypes.bfloat16)
    return hi, lo


def _prep(h, adj, w, a_src, a_dst, b):
    h = np.asarray(h, dtype=np.float32)
    adj = np.asarray(adj)
    w = np.asarray(w, dtype=np.float32)
    a_src = np.asarray(a_src, dtype=np.float32)
    a_dst = np.asarray(a_dst, dtype=np.float32)
    b = np.asarray(b, dtype=np.float32)

    perms, LO, HI = _compute_layout(h, w, a_src, a_dst)
    add_b = bool(np.any(b != 0.0))

    eye3 = np.eye(3, dtype=np.float32)
    eye65 = np.eye(VW, dtype=np.float32)
    ones1 = np.ones((1, 128), dtype=np.float32).astype(ml_dtypes.bfloat16)
    b_row = b[None, :].astype(ml_dtypes.bfloat16)
    hT = np.ascontiguousarray(h.T)  # fp32 [F_IN, N]

    in_maps = []
    for c in range(N_HEAD):
        pi, pj = perms[c]
        wa_s = (w[c] @ a_src[c])[:, 0]
        wa_d = (w[c] @ a_dst[c])[:, 0]
        # wa6 cols: 0=d_hi 1=d5_hi 2=d_lo 3=d5_lo 4=s_hi 5=s_lo
        dh, dl = _split_hilo(wa_d)
        d5h, d5l = _split_hilo(NEG * wa_d)
        sh, sl_ = _split_hilo(wa_s)
        wa6 = np.stack([x.astype(np.float32) for x in
                        (dh, d5h, dl, d5l, sh, sl_)], axis=1)
        in_maps.append({
            "hTi": np.ascontiguousarray(hT[:, pi]).astype(ml_dtypes.bfloat16),
            "hTj": np.ascontiguousarray(hT[:, pj]).astype(ml_dtypes.bfloat16),
            "wa6": np.ascontiguousarray(wa6).astype(ml_dtypes.bfloat16),
            "w_bf": np.ascontiguousarray(w[c]).astype(ml_dtypes.bfloat16),
            "eye3": eye3,
            "eye65": eye65,
            "ones1": ones1,
            "b_row": b_row,
            "mask8": np.ascontiguousarray(
                adj.T[np.ix_(pj, pi)]).astype(ml_dtypes.float8_e4m3),
        })
    return in_maps, perms, LO, HI, add_b


def _run(in_maps, LO, HI, add_b, trace=False, **kwargs):
    nc = _get_nc(LO, HI, add_b)
    return run_bass_kernel_spmd(nc, in_maps, list(range(N_HEAD)), trace=trace,
                                **kwargs)


def kernel(h, adj, w, a_src, a_dst, b):
    in_maps, perms, LO, HI, add_b = _prep(h, adj, w, a_src, a_dst, b)
    res = _run(in_maps, LO, HI, add_b)
    out = np.empty((N_HEAD, N, F_OUT), np.float32)
    for c in range(N_HEAD):
        pi, _ = perms[c]
        out[c][pi] = res.results[c]["out_nat"]
    return out
